# revision 30
# baseline (speedup 1.0000x reference)
"""Multi-head attention (B=8, H=8, S=1024, d=128) on 8 TRN2 NeuronCores.

Strategy
--------
- 2D sharding over (batch, head): the 64 (batch, head) attention
  problems are dealt to the 8 cores so that every core gets the same
  mix of "large-mask" and "small-mask" batches (the number of 128-wide
  key tiles surviving seq_mask compaction varies per batch, and the
  scalar engine's exp throughput is the kernel bottleneck). Each core
  runs n_a head-slots with kt_a key tiles and n_b = 8-n_a slots with
  kt_b tiles; the (kt_a, n_a, kt_b) program shape is identical on all
  cores (SPMD), only the data differs.
- Host-side prep (layout only): per batch, compact keys/values to the
  seq_mask-selected rows (zero-padded to the segment's k-tile count),
  pre-transpose Q and compacted K so the contraction dim (d) lands on
  SBUF partitions, and cast matmul operands to fp16. V is augmented per
  head with a 129th "indicator" column (1 for real keys, 0 for padding)
  so the softmax denominator falls out of the AV matmul.
- Device math per head-slot:
    logitsT[k, q] = K_h^T.T @ Q_h^T          (PE, M=128 k-tiles, N=512)
    W^T[k, q]     = exp(logitsT * d^-0.5)    (ACT, PSUM -> SBUF fp16,
                                              batched in alternating
                                              1536/1024-col groups to
                                              amortize ACTIVATE overhead)
    out[q, 129]   = sum_kt W^T[kt,qtile].T @ [V_h[kt] | ind[kt]]
                                             (PE, M=128 q-tiles, N=129,
                                              PSUM accumulation over kt;
                                              col 128 = denominator)
    osb[q, d]     = out[:, :128] * recip(out[:, 128])  (DVE)
  The learned scalar bias b cancels in softmax (shift invariance) and
  the -1e30 masking is equivalent to dropping masked keys, which the
  compaction does exactly.
- Software pipelining: AV+epilogue of slot s-1 are interleaved into the
  QK group stream of slot s so the scalar engine never waits at slot
  boundaries. A short burst of dummy matmuls at kernel start warms the
  PE HAM clock gate while the first input DMAs are in flight.
- Output per head-slot is DMA'd as a contiguous [128, 1024] fp16 block
  ([q-within-tile, (q-tile, d)]); the host reassembles [S, D] and
  handles the degenerate all-masked batch (uniform average).
"""
from contextlib import ExitStack

import numpy as np

import concourse.bacc as bacc
import concourse.mybir as mybir
import concourse.tile as tile
from concourse.bass_utils import run_bass_kernel_spmd

F32 = mybir.dt.float32
F16 = mybir.dt.float16
Exp = mybir.ActivationFunctionType.Exp

B, S, D, H = 8, 1024, 1024, 8
DH = D // H              # 128, head dim = one partition tile
SCALE = float(DH) ** -0.5
NQT = S // 128           # 8 q-tiles per head

_NC_CACHE: dict[tuple, object] = {}

# build options (overridable for profiling experiments)
OPTS: dict = {}


def _exp_groups(total, force_even=False):
    """Alternating 1536/1024-col exp groups covering `total` columns.

    With force_even, an odd group count ending in a full 1536 A-group is
    reshaped to end on a B-group (512-A + 1024-B) so that consecutive
    slots keep strict A/B alternation (no same-slot-pool stall at the
    slot boundary)."""
    groups, pos, a = [], 0, True
    while pos < total:
        size = min(1536 if a else 1024, total - pos)
        groups.append((a, pos, size))
        pos += size
        a = not a
    if force_even and len(groups) % 2 and groups[-1][2] == 1536:
        a_, p, _ = groups.pop()
        groups += [(a_, p, 512), (not a_, p + 512, 1024)]
    return groups


def _build(kt_a: int, kt_b: int, n_a: int, opts: dict | None = None):
    """Per-core kernel: n_a head-slots with kt_a k-tiles, then 8-n_a
    slots with kt_b k-tiles."""
    opts = opts or {}
    w_bufs = opts.get("w_bufs", 2)
    o_bufs = opts.get("o_bufs", 2)
    n_warm = opts.get("n_warm", 8)
    n_b = H - n_a
    KPA, KPB = kt_a * 128, kt_b * 128
    VWA, VWB = n_a * 129, n_b * 129
    nc = bacc.Bacc("TRN2", target_bir_lowering=False, debug=False)

    q_t = nc.dram_tensor("q_t", [D, S], F16, kind="ExternalInput")
    k_a = nc.dram_tensor("k_a", [n_a * DH, KPA], F16, kind="ExternalInput")
    k_b = nc.dram_tensor("k_b", [n_b * DH, KPB], F16, kind="ExternalInput")
    v_a = nc.dram_tensor("v_a", [KPA, VWA], F16, kind="ExternalInput")
    v_b = nc.dram_tensor("v_b", [KPB, VWB], F16, kind="ExternalInput")
    out_t = nc.dram_tensor("out_t", [H, 128, S], F16, kind="ExternalOutput")

    def po_off(qi):
        g, j = divmod(qi, 3)
        return g * 512 + j * 129

    with tile.TileContext(nc) as tc, ExitStack() as ctx:
        sb_k = ctx.enter_context(tc.tile_pool(name="sb_k", bufs=1))
        sb_q = ctx.enter_context(tc.tile_pool(name="sb_q", bufs=1))
        sb_v = ctx.enter_context(tc.tile_pool(name="sb_v", bufs=1))
        sb_wm = ctx.enter_context(tc.tile_pool(name="sb_wm", bufs=1))
        sb_w = ctx.enter_context(tc.tile_pool(name="sb_w", bufs=w_bufs))
        sb_o = ctx.enter_context(tc.tile_pool(name="sb_o", bufs=o_bufs))
        ps_a = ctx.enter_context(tc.tile_pool(name="ps_a", bufs=1, space="PSUM"))
        ps_b = ctx.enter_context(tc.tile_pool(name="ps_b", bufs=1, space="PSUM"))
        ps_o = ctx.enter_context(tc.tile_pool(name="ps_o", bufs=1, space="PSUM"))

        kalls = [sb_k.tile([128, n_a * KPA], F16, tag="ka", name="ka"),
                 sb_k.tile([128, n_b * KPB], F16, tag="kb", name="kb")]
        qall = sb_q.tile([128, H * S], F16)
        valls = [sb_v.tile([128, kt_a * VWA], F16, tag="va", name="va"),
                 sb_v.tile([128, kt_b * VWB], F16, tag="vb", name="vb")]
        kts = [k_a, k_b]
        vts = [v_a, v_b]
        KPs, VWs = (KPA, KPB), (VWA, VWB)

        # --- PE warmup: dense dummy matmuls while the first DMAs fly, so
        # the HAM clock gate reaches 8/8 before real work arrives.
        if n_warm:
            wl = sb_wm.tile([128, 128], F16)
            wr = sb_wm.tile([128, 512], F16)
            nc.gpsimd.memset(wl[:], 0.0)
            nc.gpsimd.memset(wr[:], 0.0)
            warm_po = ps_o.tile([128, 1536], F32, tag="po", name="po_warm")
            for _ in range(n_warm):
                nc.tensor.matmul(warm_po[:, 0:512], wl[:], wr[:],
                                 start=True, stop=True, skip_group_check=True)

        # --- Input DMAs, split per head-slot / per k-tile; the first few
        # are issued from different engines so their transfers overlap.
        def dma_k(seg, si, eng):
            KP = KPs[seg]
            eng.dma_start(kalls[seg][:, si * KP:(si + 1) * KP],
                          kts[seg].ap()[si * DH:(si + 1) * DH, :])

        def dma_q(s, eng):
            eng.dma_start(
                qall[:, s * S:(s + 1) * S], q_t.ap()[s * DH:(s + 1) * DH, :])

        def dma_v(seg, kt, eng):
            VW = VWs[seg]
            eng.dma_start(
                valls[seg][:, kt * VW:(kt + 1) * VW],
                vts[seg].ap()[kt * 128:(kt + 1) * 128, :])

        dma_q(0, nc.sync)
        dma_k(0, 0, nc.gpsimd)
        dma_k(0, 1, nc.scalar)
        dma_q(1, nc.sync)
        dma_v(0, 0, nc.scalar)
        dma_v(0, 1, nc.scalar)
        dma_q(2, nc.sync)
        for kt in range(2, kt_a):
            dma_v(0, kt, nc.gpsimd)
        for si in range(2, n_a):
            dma_k(0, si, nc.gpsimd)
            if si >= 3:
                dma_q(si, nc.sync)
        for si in range(n_b):
            dma_k(1, si, nc.gpsimd)
        for s in range(n_a, H):
            dma_q(s, nc.sync)
        for kt in range(kt_b):
            dma_v(1, kt, nc.sync)

        def emit_qk_group(s, gi, ring, groups):
            seg = 0 if s < n_a else 1
            si = s if s < n_a else s - n_a
            KP = KPs[seg]
            a, start, size = groups[gi]
            pool = ps_a if a else ps_b
            cap = 1536 if a else 1024
            pl = pool.tile([128, cap], F32, tag="pl" + ("A" if a else "B"),
                           name=f"pl_{s}_{start}")
            for local in range(0, size, 512):
                gcol = start + local
                kt, qh = divmod(gcol, 1024)
                lhsT = kalls[seg][:, si * KP + kt * 128: si * KP + (kt + 1) * 128]
                nc.tensor.matmul(
                    pl[:, local:local + 512],
                    lhsT, qall[:, s * S + qh:s * S + qh + 512],
                    start=True, stop=True)
            nc.scalar.activation(
                ring[:, start:start + size], pl[:, 0:size], Exp, scale=SCALE)

        def emit_av_kt(s, ring, kt, po, n_kt):
            seg = 0 if s < n_a else 1
            si = s if s < n_a else s - n_a
            VW = VWs[seg]
            first, last = kt == 0, kt == n_kt - 1
            rhs = valls[seg][:, kt * VW + si * 129: kt * VW + (si + 1) * 129]
            for qi in range(NQT):
                off = po_off(qi)
                # start=True clears the has_written bits of the WHOLE
                # bank, so only the first matmul touching each bank may
                # carry it; the other regions' first writes rely on
                # their (now cleared) bits selecting overwrite mode.
                nc.tensor.matmul(
                    po[:, off:off + 129],
                    ring[:, kt * 1024 + qi * 128: kt * 1024 + (qi + 1) * 128],
                    rhs, start=first and qi % 3 == 0, stop=last,
                    skip_group_check=True)

        def emit_epilogue(s, po, last=False):
            # Split per PSUM bank; the copies run first since they are
            # what releases the po banks for the next slot's AV matmuls.
            oal = sb_o.tile([128, 1536], F16, tag="oal", name=f"oal_{s}")
            rst = sb_o.tile([128, 9], F32, tag="rst", name=f"rst_{s}")
            osb = sb_o.tile([128, S], F16, tag="osb", name=f"osb_{s}")
            for g in range(3):
                cnt = 3 if g < 2 else NQT - 6
                base = g * 512
                nc.vector.tensor_copy(
                    oal[:, base:base + cnt * 129], po[:, base:base + cnt * 129])
            # one strided reciprocal covers all three bank-groups' den
            # columns ([128, 3, 3]; the (2,2) element is unused padding)
            nc.vector.reciprocal(
                rst[:], oal.rearrange("p (g x) -> p g x", g=3)[:, :, 128:512:129])
            for qi in range(NQT):
                g, j = divmod(qi, 3)
                nc.vector.tensor_scalar_mul(
                    osb[:, qi * 128:(qi + 1) * 128],
                    oal[:, g * 512 + j * 129:g * 512 + j * 129 + 128],
                    rst[:, g * 3 + j:g * 3 + j + 1])
                if last and qi == 3:
                    # stream the first half out while the rest normalizes
                    nc.gpsimd.dma_start(out_t.ap()[s][:, 0:512], osb[:, 0:512])
            if last:
                nc.gpsimd.dma_start(out_t.ap()[s][:, 512:S], osb[:, 512:S])
            else:
                nc.gpsimd.dma_start(out_t.ap()[s], osb[:])

        # Boundary-level software pipeline. Per slot s the PE queue gets:
        #   QK(s, g0) | AV(s-1, kt 0..last-1) | QK(s, g1) | AV(s-1, last)
        #   | QK(s, g2..) | epilogue(s-1)
        # so exp(s, g0) starts the moment exp(s-1, last) finishes, and the
        # previous slot's AV (whose exps are long done) fills PE idle time
        # without head-of-line blocking the QK stream.
        n_kts = [kt_a] * n_a + [kt_b] * n_b
        rings, pos, grps = {}, {}, {}
        for s in range(H):
            n_kt = n_kts[s]
            grps[s] = _exp_groups(n_kt * 1024, force_even=s < H - 1)
            rings[s] = sb_w.tile([128, n_kt * 1024], F16, tag="ring",
                                 name=f"ring_{s}")
            if s >= 1:
                pos[s - 1] = ps_o.tile([128, 1536], F32, tag="po",
                                       name=f"po_{s - 1}")
            for gi in range(len(grps[s])):
                emit_qk_group(s, gi, rings[s], grps[s])
                if s >= 1 and gi == 0:
                    for kt in range(n_kts[s - 1] - 1):
                        emit_av_kt(s - 1, rings[s - 1], kt, pos[s - 1],
                                   n_kts[s - 1])
                if s >= 1 and gi == 1:
                    emit_av_kt(s - 1, rings[s - 1], n_kts[s - 1] - 1,
                               pos[s - 1], n_kts[s - 1])
            if s >= 1:
                emit_epilogue(s - 1, pos.pop(s - 1))
                rings.pop(s - 1)
        # Last slot: its AV matmuls chase the exps straight down the queue.
        pos[H - 1] = ps_o.tile([128, 1536], F32, tag="po", name=f"po_{H - 1}")
        for kt in range(n_kts[H - 1]):
            emit_av_kt(H - 1, rings[H - 1], kt, pos[H - 1], n_kts[H - 1])
        emit_epilogue(H - 1, pos.pop(H - 1), last=True)

    nc.compile()
    return nc


def kernel(memory, query, seq_mask, b):
    memory = np.ascontiguousarray(memory, dtype=np.float32)
    query = np.ascontiguousarray(query, dtype=np.float32)
    seq_mask = np.asarray(seq_mask)
    assert memory.shape == (B, S, 2 * D) and query.shape == (B, S, D)

    counts = [int(np.count_nonzero(seq_mask[i])) for i in range(B)]
    tiles = [max((c + 127) // 128, 1) for c in counts]
    kt_a = max(tiles)
    big = [i for i in range(B) if tiles[i] == kt_a]
    if len(big) == B:
        big = big[:-1]
    small = [i for i in range(B) if i not in big]
    kt_b = max(tiles[i] for i in small)
    n_a = len(big)            # head-slots per core from "big" batches

    key = (kt_a, kt_b, n_a, tuple(sorted(OPTS.items())))
    if key not in _NC_CACHE:
        _NC_CACHE[key] = _build(kt_a, kt_b, n_a, OPTS)
    nc = _NC_CACHE[key]

    # Per-batch compacted/transposed operands (fp16).
    q_t = np.ascontiguousarray(query.transpose(0, 2, 1)).astype(np.float16)
    ktb_all, vab_all = {}, {}
    for i in range(B):
        kp = (kt_a if i in big else kt_b) * 128
        idx = np.flatnonzero(seq_mask[i])
        nb = len(idx)
        ktb = np.zeros((D, kp), dtype=np.float16)
        vab = np.zeros((kp, H, 129), dtype=np.float16)
        if nb:
            ktb[:, :nb] = memory[i, idx, :D].T
            vab[:nb, :, :128] = memory[i, idx, D:].reshape(nb, H, DH)
            vab[:nb, :, 128] = 1.0
        ktb_all[i], vab_all[i] = ktb, vab

    # Deal the (batch, head) pairs: core c gets n_a consecutive entries
    # of the "big" head list and 8-n_a of the "small" head list.
    heads_a = [(bi, h) for bi in big for h in range(H)]
    heads_b = [(bi, h) for bi in small for h in range(H)]
    n_b = H - n_a
    placements, in_maps = [], []
    for c in range(B):
        slots = heads_a[c * n_a:(c + 1) * n_a] + heads_b[c * n_b:(c + 1) * n_b]
        placements.append(slots)
        qrows = np.concatenate(
            [q_t[bi][h * DH:(h + 1) * DH] for bi, h in slots], axis=0)
        ka = np.concatenate(
            [ktb_all[bi][h * DH:(h + 1) * DH] for bi, h in slots[:n_a]], axis=0)
        kb = np.concatenate(
            [ktb_all[bi][h * DH:(h + 1) * DH] for bi, h in slots[n_a:]], axis=0)
        va = np.concatenate(
            [vab_all[bi][:, h] for bi, h in slots[:n_a]], axis=1)
        vb = np.concatenate(
            [vab_all[bi][:, h] for bi, h in slots[n_a:]], axis=1)
        in_maps.append({
            "q_t": np.ascontiguousarray(qrows),
            "k_a": np.ascontiguousarray(ka),
            "k_b": np.ascontiguousarray(kb),
            "v_a": np.ascontiguousarray(va),
            "v_b": np.ascontiguousarray(vb),
        })

    res = run_bass_kernel_spmd(nc, in_maps, list(range(B)))
    out = np.empty((B, S, D), dtype=np.float32)
    for c, slots in enumerate(placements):
        o = res.results[c]["out_t"].astype(np.float32)   # [8, 128, S]
        for j, (bi, h) in enumerate(slots):
            # [p, (qi d)] -> [qi, p, d] -> [S, d]
            blk = o[j].reshape(128, NQT, DH).transpose(1, 0, 2)
            out[bi][:, h * DH:(h + 1) * DH] = blk.reshape(S, DH)
    for i in range(B):
        if counts[i] == 0:
            # all keys masked: reference softmax degenerates to uniform
            out[i] = memory[i, :, D:].mean(axis=0)[None, :]
    return out


# revision 32
# speedup vs baseline: 1.1811x; 1.1811x over previous
"""Multi-head attention (B=8, H=8, S=1024, d=128) on 8 TRN2 NeuronCores.

Strategy
--------
- 2D sharding over (batch, head): the 64 (batch, head) attention
  problems are dealt to the 8 cores so that every core gets the same
  mix of "large-mask" and "small-mask" batches (the number of 128-wide
  key tiles surviving seq_mask compaction varies per batch, and the
  scalar engine's exp throughput is the kernel bottleneck). Each core
  runs n_a head-slots with kt_a key tiles and n_b = 8-n_a slots with
  kt_b tiles; the (kt_a, n_a, kt_b) program shape is identical on all
  cores (SPMD), only the data differs.
- Host-side prep (layout only): per batch, compact keys/values to the
  seq_mask-selected rows (zero-padded to the segment's k-tile count),
  pre-transpose Q and compacted K so the contraction dim (d) lands on
  SBUF partitions, and cast matmul operands to fp16. V is augmented per
  head with a 129th "indicator" column (1 for real keys, 0 for padding)
  so the softmax denominator falls out of the AV matmul.
- Device math per head-slot:
    logitsT[k, q] = K_h^T.T @ Q_h^T          (PE, M=128 k-tiles, N=512)
    W^T[k, q]     = exp(logitsT * d^-0.5)    (ACT, PSUM -> SBUF fp16,
                                              batched in alternating
                                              1536/1024-col groups to
                                              amortize ACTIVATE overhead)
    out[q, 129]   = sum_kt W^T[kt,qtile].T @ [V_h[kt] | ind[kt]]
                                             (PE, M=128 q-tiles, N=129,
                                              PSUM accumulation over kt;
                                              col 128 = denominator)
    osb[q, d]     = out[:, :128] * recip(out[:, 128])  (DVE)
  The learned scalar bias b cancels in softmax (shift invariance) and
  the -1e30 masking is equivalent to dropping masked keys, which the
  compaction does exactly.
- Software pipelining: AV+epilogue of slot s-1 are interleaved into the
  QK group stream of slot s so the scalar engine never waits at slot
  boundaries. A short burst of dummy matmuls at kernel start warms the
  PE HAM clock gate while the first input DMAs are in flight.
- Output per head-slot is DMA'd as a contiguous [128, 1024] fp16 block
  ([q-within-tile, (q-tile, d)]); the host reassembles [S, D] and
  handles the degenerate all-masked batch (uniform average).
"""
from contextlib import ExitStack

import numpy as np

import concourse.bacc as bacc
import concourse.mybir as mybir
import concourse.tile as tile
from concourse.bass_utils import run_bass_kernel_spmd

F32 = mybir.dt.float32
F16 = mybir.dt.float16
Exp = mybir.ActivationFunctionType.Exp

B, S, D, H = 8, 1024, 1024, 8
DH = D // H              # 128, head dim = one partition tile
SCALE = float(DH) ** -0.5
NQT = S // 128           # 8 q-tiles per head

_NC_CACHE: dict[tuple, object] = {}

# build options (overridable for profiling experiments)
OPTS: dict = {}


def _exp_groups(total, force_even=False):
    """Alternating 1536/1024-col exp groups covering `total` columns.

    With force_even, an odd group count ending in a full 1536 A-group is
    reshaped to end on a B-group (512-A + 1024-B) so that consecutive
    slots keep strict A/B alternation (no same-slot-pool stall at the
    slot boundary)."""
    groups, pos, a = [], 0, True
    while pos < total:
        size = min(1536 if a else 1024, total - pos)
        groups.append((a, pos, size))
        pos += size
        a = not a
    if force_even and len(groups) % 2 and groups[-1][2] == 1536:
        a_, p, _ = groups.pop()
        groups += [(a_, p, 512), (not a_, p + 512, 1024)]
    return groups


def _build(kt_a: int, kt_b: int, n_a: int, opts: dict | None = None):
    """Per-core kernel: n_a head-slots with kt_a k-tiles, then 8-n_a
    slots with kt_b k-tiles."""
    opts = opts or {}
    w_bufs = opts.get("w_bufs", 2)
    o_bufs = opts.get("o_bufs", 2)
    n_warm = opts.get("n_warm", 8)
    n_b = H - n_a
    KPA, KPB = kt_a * 128, kt_b * 128
    VWA, VWB = n_a * 129, n_b * 129
    nc = bacc.Bacc("TRN2", target_bir_lowering=False, debug=False)

    q_t = nc.dram_tensor("q_t", [D, S], F16, kind="ExternalInput")
    k_a = nc.dram_tensor("k_a", [n_a * DH, KPA], F16, kind="ExternalInput")
    k_b = nc.dram_tensor("k_b", [n_b * DH, KPB], F16, kind="ExternalInput")
    v_a = nc.dram_tensor("v_a", [KPA, VWA], F16, kind="ExternalInput")
    v_b = nc.dram_tensor("v_b", [KPB, VWB], F16, kind="ExternalInput")
    out_t = nc.dram_tensor("out_t", [H, 128, S], F16, kind="ExternalOutput")

    def po_off(qi):
        g, j = divmod(qi, 3)
        return g * 512 + j * 129

    with tile.TileContext(nc) as tc, ExitStack() as ctx:
        sb_k = ctx.enter_context(tc.tile_pool(name="sb_k", bufs=1))
        sb_q = ctx.enter_context(tc.tile_pool(name="sb_q", bufs=1))
        sb_v = ctx.enter_context(tc.tile_pool(name="sb_v", bufs=1))
        sb_wm = ctx.enter_context(tc.tile_pool(name="sb_wm", bufs=1))
        sb_w = ctx.enter_context(tc.tile_pool(name="sb_w", bufs=w_bufs))
        sb_o = ctx.enter_context(tc.tile_pool(name="sb_o", bufs=o_bufs))
        ps_a = ctx.enter_context(tc.tile_pool(name="ps_a", bufs=1, space="PSUM"))
        ps_b = ctx.enter_context(tc.tile_pool(name="ps_b", bufs=1, space="PSUM"))
        ps_o = ctx.enter_context(tc.tile_pool(name="ps_o", bufs=1, space="PSUM"))

        kalls = [sb_k.tile([128, n_a * KPA], F16, tag="ka", name="ka"),
                 sb_k.tile([128, n_b * KPB], F16, tag="kb", name="kb")]
        qall = sb_q.tile([128, H * S], F16)
        valls = [sb_v.tile([128, kt_a * VWA], F16, tag="va", name="va"),
                 sb_v.tile([128, kt_b * VWB], F16, tag="vb", name="vb")]
        kts = [k_a, k_b]
        vts = [v_a, v_b]
        KPs, VWs = (KPA, KPB), (VWA, VWB)

        # --- PE warmup: dense dummy matmuls while the first DMAs fly, so
        # the HAM clock gate reaches 8/8 before real work arrives.
        if n_warm:
            wl = sb_wm.tile([128, 128], F16)
            wr = sb_wm.tile([128, 512], F16)
            nc.gpsimd.memset(wl[:], 0.0)
            nc.gpsimd.memset(wr[:], 0.0)
            warm_po = ps_o.tile([128, 1536], F32, tag="po", name="po_warm")
            for _ in range(n_warm):
                nc.tensor.matmul(warm_po[:, 0:512], wl[:], wr[:],
                                 start=True, stop=True, skip_group_check=True)

        # --- Input DMAs, split per head-slot / per k-tile; the first few
        # are issued from different engines so their transfers overlap.
        def dma_k(seg, si, eng):
            KP = KPs[seg]
            eng.dma_start(kalls[seg][:, si * KP:(si + 1) * KP],
                          kts[seg].ap()[si * DH:(si + 1) * DH, :])

        def dma_q(s, eng):
            eng.dma_start(
                qall[:, s * S:(s + 1) * S], q_t.ap()[s * DH:(s + 1) * DH, :])

        def dma_v(seg, kt, eng):
            VW = VWs[seg]
            eng.dma_start(
                valls[seg][:, kt * VW:(kt + 1) * VW],
                vts[seg].ap()[kt * 128:(kt + 1) * 128, :])

        dma_q(0, nc.sync)
        dma_k(0, 0, nc.gpsimd)
        dma_k(0, 1, nc.scalar)
        dma_q(1, nc.sync)
        dma_v(0, 0, nc.scalar)
        dma_v(0, 1, nc.scalar)
        dma_q(2, nc.sync)
        for kt in range(2, kt_a):
            dma_v(0, kt, nc.gpsimd)
        for si in range(2, n_a):
            dma_k(0, si, nc.gpsimd)
            if si >= 3:
                dma_q(si, nc.sync)
        for si in range(n_b):
            dma_k(1, si, nc.gpsimd)
        for s in range(n_a, H):
            dma_q(s, nc.sync)
        for kt in range(kt_b):
            dma_v(1, kt, nc.sync)

        def emit_qk_group(s, gi, ring, groups):
            seg = 0 if s < n_a else 1
            si = s if s < n_a else s - n_a
            KP = KPs[seg]
            a, start, size = groups[gi]
            pool = ps_a if a else ps_b
            cap = 1536 if a else 1024
            pl = pool.tile([128, cap], F32, tag="pl" + ("A" if a else "B"),
                           name=f"pl_{s}_{start}")
            for local in range(0, size, 512):
                gcol = start + local
                kt, qh = divmod(gcol, 1024)
                lhsT = kalls[seg][:, si * KP + kt * 128: si * KP + (kt + 1) * 128]
                nc.tensor.matmul(
                    pl[:, local:local + 512],
                    lhsT, qall[:, s * S + qh:s * S + qh + 512],
                    start=True, stop=True)
            nc.scalar.activation(
                ring[:, start:start + size], pl[:, 0:size], Exp, scale=SCALE)

        def emit_av_kt(s, ring, kt, po, n_kt):
            seg = 0 if s < n_a else 1
            si = s if s < n_a else s - n_a
            VW = VWs[seg]
            first, last = kt == 0, kt == n_kt - 1
            rhs = valls[seg][:, kt * VW + si * 129: kt * VW + (si + 1) * 129]
            for qi in range(NQT):
                off = po_off(qi)
                # start=True clears the has_written bits of the WHOLE
                # bank, so only the first matmul touching each bank may
                # carry it; the other regions' first writes rely on
                # their (now cleared) bits selecting overwrite mode.
                nc.tensor.matmul(
                    po[:, off:off + 129],
                    ring[:, kt * 1024 + qi * 128: kt * 1024 + (qi + 1) * 128],
                    rhs, start=first and qi % 3 == 0, stop=last,
                    skip_group_check=True)

        def emit_epilogue(s, po, last=False):
            # Split per PSUM bank; the copies run first since they are
            # what releases the po banks for the next slot's AV matmuls.
            oal = sb_o.tile([128, 1536], F16, tag="oal", name=f"oal_{s}")
            rst = sb_o.tile([128, 9], F32, tag="rst", name=f"rst_{s}")
            osb = sb_o.tile([128, S], F16, tag="osb", name=f"osb_{s}")
            for g in range(3):
                cnt = 3 if g < 2 else NQT - 6
                base = g * 512
                nc.vector.tensor_copy(
                    oal[:, base:base + cnt * 129], po[:, base:base + cnt * 129])
            for g in range(3):
                cnt = 3 if g < 2 else NQT - 6
                base = g * 512
                nc.vector.reciprocal(
                    rst[:, g * 3:g * 3 + cnt],
                    oal[:, base + 128:base + cnt * 129:129])
            for qi in range(NQT):
                g, j = divmod(qi, 3)
                nc.vector.tensor_scalar_mul(
                    osb[:, qi * 128:(qi + 1) * 128],
                    oal[:, g * 512 + j * 129:g * 512 + j * 129 + 128],
                    rst[:, qi:qi + 1])
            nc.gpsimd.dma_start(out_t.ap()[s], osb[:])

        # Boundary-level software pipeline. Per slot s the PE queue gets:
        #   QK(s, g0) | AV(s-1, kt 0..last-1) | QK(s, g1) | AV(s-1, last)
        #   | QK(s, g2..) | epilogue(s-1)
        # so exp(s, g0) starts the moment exp(s-1, last) finishes, and the
        # previous slot's AV (whose exps are long done) fills PE idle time
        # without head-of-line blocking the QK stream.
        n_kts = [kt_a] * n_a + [kt_b] * n_b
        rings, pos, grps = {}, {}, {}
        for s in range(H):
            n_kt = n_kts[s]
            grps[s] = _exp_groups(n_kt * 1024, force_even=s < H - 1)
            rings[s] = sb_w.tile([128, n_kt * 1024], F16, tag="ring",
                                 name=f"ring_{s}")
            if s >= 1:
                pos[s - 1] = ps_o.tile([128, 1536], F32, tag="po",
                                       name=f"po_{s - 1}")
            for gi in range(len(grps[s])):
                emit_qk_group(s, gi, rings[s], grps[s])
                if s >= 1 and gi == 0:
                    for kt in range(n_kts[s - 1] - 1):
                        emit_av_kt(s - 1, rings[s - 1], kt, pos[s - 1],
                                   n_kts[s - 1])
                if s >= 1 and gi == 1:
                    emit_av_kt(s - 1, rings[s - 1], n_kts[s - 1] - 1,
                               pos[s - 1], n_kts[s - 1])
            if s >= 1:
                emit_epilogue(s - 1, pos.pop(s - 1))
                rings.pop(s - 1)
        # Last slot: its AV matmuls chase the exps straight down the queue.
        pos[H - 1] = ps_o.tile([128, 1536], F32, tag="po", name=f"po_{H - 1}")
        for kt in range(n_kts[H - 1]):
            emit_av_kt(H - 1, rings[H - 1], kt, pos[H - 1], n_kts[H - 1])
        emit_epilogue(H - 1, pos.pop(H - 1), last=True)

    nc.compile()
    return nc


def kernel(memory, query, seq_mask, b):
    memory = np.ascontiguousarray(memory, dtype=np.float32)
    query = np.ascontiguousarray(query, dtype=np.float32)
    seq_mask = np.asarray(seq_mask)
    assert memory.shape == (B, S, 2 * D) and query.shape == (B, S, D)

    counts = [int(np.count_nonzero(seq_mask[i])) for i in range(B)]
    tiles = [max((c + 127) // 128, 1) for c in counts]
    kt_a = max(tiles)
    big = [i for i in range(B) if tiles[i] == kt_a]
    if len(big) == B:
        big = big[:-1]
    small = [i for i in range(B) if i not in big]
    kt_b = max(tiles[i] for i in small)
    n_a = len(big)            # head-slots per core from "big" batches

    key = (kt_a, kt_b, n_a, tuple(sorted(OPTS.items())))
    if key not in _NC_CACHE:
        _NC_CACHE[key] = _build(kt_a, kt_b, n_a, OPTS)
    nc = _NC_CACHE[key]

    # Per-batch compacted/transposed operands (fp16).
    q_t = np.ascontiguousarray(query.transpose(0, 2, 1)).astype(np.float16)
    ktb_all, vab_all = {}, {}
    for i in range(B):
        kp = (kt_a if i in big else kt_b) * 128
        idx = np.flatnonzero(seq_mask[i])
        nb = len(idx)
        ktb = np.zeros((D, kp), dtype=np.float16)
        vab = np.zeros((kp, H, 129), dtype=np.float16)
        if nb:
            ktb[:, :nb] = memory[i, idx, :D].T
            vab[:nb, :, :128] = memory[i, idx, D:].reshape(nb, H, DH)
            vab[:nb, :, 128] = 1.0
        ktb_all[i], vab_all[i] = ktb, vab

    # Deal the (batch, head) pairs: core c gets n_a consecutive entries
    # of the "big" head list and 8-n_a of the "small" head list.
    heads_a = [(bi, h) for bi in big for h in range(H)]
    heads_b = [(bi, h) for bi in small for h in range(H)]
    n_b = H - n_a
    placements, in_maps = [], []
    for c in range(B):
        slots = heads_a[c * n_a:(c + 1) * n_a] + heads_b[c * n_b:(c + 1) * n_b]
        placements.append(slots)
        qrows = np.concatenate(
            [q_t[bi][h * DH:(h + 1) * DH] for bi, h in slots], axis=0)
        ka = np.concatenate(
            [ktb_all[bi][h * DH:(h + 1) * DH] for bi, h in slots[:n_a]], axis=0)
        kb = np.concatenate(
            [ktb_all[bi][h * DH:(h + 1) * DH] for bi, h in slots[n_a:]], axis=0)
        va = np.concatenate(
            [vab_all[bi][:, h] for bi, h in slots[:n_a]], axis=1)
        vb = np.concatenate(
            [vab_all[bi][:, h] for bi, h in slots[n_a:]], axis=1)
        in_maps.append({
            "q_t": np.ascontiguousarray(qrows),
            "k_a": np.ascontiguousarray(ka),
            "k_b": np.ascontiguousarray(kb),
            "v_a": np.ascontiguousarray(va),
            "v_b": np.ascontiguousarray(vb),
        })

    res = run_bass_kernel_spmd(nc, in_maps, list(range(B)))
    out = np.empty((B, S, D), dtype=np.float32)
    for c, slots in enumerate(placements):
        o = res.results[c]["out_t"].astype(np.float32)   # [8, 128, S]
        for j, (bi, h) in enumerate(slots):
            # [p, (qi d)] -> [qi, p, d] -> [S, d]
            blk = o[j].reshape(128, NQT, DH).transpose(1, 0, 2)
            out[bi][:, h * DH:(h + 1) * DH] = blk.reshape(S, DH)
    for i in range(B):
        if counts[i] == 0:
            # all keys masked: reference softmax degenerates to uniform
            out[i] = memory[i, :, D:].mean(axis=0)[None, :]
    return out


# revision 33
# speedup vs baseline: 1.1814x; 1.0003x over previous
"""Multi-head attention (B=8, H=8, S=1024, d=128) on 8 TRN2 NeuronCores.

Strategy
--------
- 2D sharding over (batch, head): the 64 (batch, head) attention
  problems are dealt to the 8 cores so that every core gets the same
  mix of "large-mask" and "small-mask" batches (the number of 128-wide
  key tiles surviving seq_mask compaction varies per batch, and the
  scalar engine's exp throughput is the kernel bottleneck). Each core
  runs n_a head-slots with kt_a key tiles and n_b = 8-n_a slots with
  kt_b tiles; the (kt_a, n_a, kt_b) program shape is identical on all
  cores (SPMD), only the data differs.
- Host-side prep (layout only): per batch, compact keys/values to the
  seq_mask-selected rows (zero-padded to the segment's k-tile count),
  pre-transpose Q and compacted K so the contraction dim (d) lands on
  SBUF partitions, and cast matmul operands to fp16. V is augmented per
  head with a 129th "indicator" column (1 for real keys, 0 for padding)
  so the softmax denominator falls out of the AV matmul.
- Device math per head-slot:
    logitsT[k, q] = K_h^T.T @ Q_h^T          (PE, M=128 k-tiles, N=512)
    W^T[k, q]     = exp(logitsT * d^-0.5)    (ACT, PSUM -> SBUF fp16,
                                              batched in alternating
                                              1536/1024-col groups to
                                              amortize ACTIVATE overhead)
    out[q, 129]   = sum_kt W^T[kt,qtile].T @ [V_h[kt] | ind[kt]]
                                             (PE, M=128 q-tiles, N=129,
                                              PSUM accumulation over kt;
                                              col 128 = denominator)
    osb[q, d]     = out[:, :128] * recip(out[:, 128])  (DVE)
  The learned scalar bias b cancels in softmax (shift invariance) and
  the -1e30 masking is equivalent to dropping masked keys, which the
  compaction does exactly.
- Software pipelining: AV+epilogue of slot s-1 are interleaved into the
  QK group stream of slot s so the scalar engine never waits at slot
  boundaries. A short burst of dummy matmuls at kernel start warms the
  PE HAM clock gate while the first input DMAs are in flight.
- Output per head-slot is DMA'd as a contiguous [128, 1024] fp16 block
  ([q-within-tile, (q-tile, d)]); the host reassembles [S, D] and
  handles the degenerate all-masked batch (uniform average).
"""
from contextlib import ExitStack

import numpy as np

import concourse.bacc as bacc
import concourse.mybir as mybir
import concourse.tile as tile
from concourse.bass_utils import run_bass_kernel_spmd

F32 = mybir.dt.float32
F16 = mybir.dt.float16
Exp = mybir.ActivationFunctionType.Exp

B, S, D, H = 8, 1024, 1024, 8
DH = D // H              # 128, head dim = one partition tile
SCALE = float(DH) ** -0.5
NQT = S // 128           # 8 q-tiles per head

_NC_CACHE: dict[tuple, object] = {}

# build options (overridable for profiling experiments)
OPTS: dict = {}


def _exp_groups(total, force_even=False):
    """Alternating 1536/1024-col exp groups covering `total` columns.

    With force_even, an odd group count ending in a full 1536 A-group is
    reshaped to end on a B-group (512-A + 1024-B) so that consecutive
    slots keep strict A/B alternation (no same-slot-pool stall at the
    slot boundary)."""
    groups, pos, a = [], 0, True
    while pos < total:
        size = min(1536 if a else 1024, total - pos)
        groups.append((a, pos, size))
        pos += size
        a = not a
    if force_even and len(groups) % 2 and groups[-1][2] == 1536:
        a_, p, _ = groups.pop()
        groups += [(a_, p, 512), (not a_, p + 512, 1024)]
    return groups


def _build(kt_a: int, kt_b: int, n_a: int, opts: dict | None = None):
    """Per-core kernel: n_a head-slots with kt_a k-tiles, then 8-n_a
    slots with kt_b k-tiles."""
    opts = opts or {}
    w_bufs = opts.get("w_bufs", 3)
    o_bufs = opts.get("o_bufs", 3)
    n_warm = opts.get("n_warm", 8)
    n_b = H - n_a
    KPA, KPB = kt_a * 128, kt_b * 128
    VWA, VWB = n_a * 129, n_b * 129
    nc = bacc.Bacc("TRN2", target_bir_lowering=False, debug=False)

    q_t = nc.dram_tensor("q_t", [D, S], F16, kind="ExternalInput")
    k_a = nc.dram_tensor("k_a", [n_a * DH, KPA], F16, kind="ExternalInput")
    k_b = nc.dram_tensor("k_b", [n_b * DH, KPB], F16, kind="ExternalInput")
    v_a = nc.dram_tensor("v_a", [KPA, VWA], F16, kind="ExternalInput")
    v_b = nc.dram_tensor("v_b", [KPB, VWB], F16, kind="ExternalInput")
    out_t = nc.dram_tensor("out_t", [H, 128, S], F16, kind="ExternalOutput")

    def po_off(qi):
        g, j = divmod(qi, 3)
        return g * 512 + j * 129

    with tile.TileContext(nc) as tc, ExitStack() as ctx:
        sb_k = ctx.enter_context(tc.tile_pool(name="sb_k", bufs=1))
        sb_q = ctx.enter_context(tc.tile_pool(name="sb_q", bufs=1))
        sb_v = ctx.enter_context(tc.tile_pool(name="sb_v", bufs=1))
        sb_wm = ctx.enter_context(tc.tile_pool(name="sb_wm", bufs=1))
        sb_w = ctx.enter_context(tc.tile_pool(name="sb_w", bufs=w_bufs))
        sb_o = ctx.enter_context(tc.tile_pool(name="sb_o", bufs=o_bufs))
        ps_a = ctx.enter_context(tc.tile_pool(name="ps_a", bufs=1, space="PSUM"))
        ps_b = ctx.enter_context(tc.tile_pool(name="ps_b", bufs=1, space="PSUM"))
        ps_o = ctx.enter_context(tc.tile_pool(name="ps_o", bufs=1, space="PSUM"))

        kalls = [sb_k.tile([128, n_a * KPA], F16, tag="ka", name="ka"),
                 sb_k.tile([128, n_b * KPB], F16, tag="kb", name="kb")]
        qall = sb_q.tile([128, H * S], F16)
        valls = [sb_v.tile([128, kt_a * VWA], F16, tag="va", name="va"),
                 sb_v.tile([128, kt_b * VWB], F16, tag="vb", name="vb")]
        kts = [k_a, k_b]
        vts = [v_a, v_b]
        KPs, VWs = (KPA, KPB), (VWA, VWB)

        # --- PE warmup: dense dummy matmuls while the first DMAs fly, so
        # the HAM clock gate reaches 8/8 before real work arrives.
        if n_warm:
            wl = sb_wm.tile([128, 128], F16)
            wr = sb_wm.tile([128, 512], F16)
            nc.gpsimd.memset(wl[:], 0.0)
            nc.gpsimd.memset(wr[:], 0.0)
            warm_po = ps_o.tile([128, 1536], F32, tag="po", name="po_warm")
            for _ in range(n_warm):
                nc.tensor.matmul(warm_po[:, 0:512], wl[:], wr[:],
                                 start=True, stop=True, skip_group_check=True)

        # --- Input DMAs, split per head-slot / per k-tile; the first few
        # are issued from different engines so their transfers overlap.
        def dma_k(seg, si, eng):
            KP = KPs[seg]
            eng.dma_start(kalls[seg][:, si * KP:(si + 1) * KP],
                          kts[seg].ap()[si * DH:(si + 1) * DH, :])

        def dma_q(s, eng):
            eng.dma_start(
                qall[:, s * S:(s + 1) * S], q_t.ap()[s * DH:(s + 1) * DH, :])

        def dma_v(seg, kt, eng):
            VW = VWs[seg]
            eng.dma_start(
                valls[seg][:, kt * VW:(kt + 1) * VW],
                vts[seg].ap()[kt * 128:(kt + 1) * 128, :])

        dma_q(0, nc.sync)
        dma_k(0, 0, nc.gpsimd)
        dma_k(0, 1, nc.scalar)
        dma_q(1, nc.sync)
        dma_v(0, 0, nc.scalar)
        dma_v(0, 1, nc.scalar)
        dma_q(2, nc.sync)
        for kt in range(2, kt_a):
            dma_v(0, kt, nc.gpsimd)
        for si in range(2, n_a):
            dma_k(0, si, nc.gpsimd)
            if si >= 3:
                dma_q(si, nc.sync)
        for si in range(n_b):
            dma_k(1, si, nc.gpsimd)
        for s in range(n_a, H):
            dma_q(s, nc.sync)
        for kt in range(kt_b):
            dma_v(1, kt, nc.sync)

        def emit_qk_group(s, gi, ring, groups):
            seg = 0 if s < n_a else 1
            si = s if s < n_a else s - n_a
            KP = KPs[seg]
            a, start, size = groups[gi]
            pool = ps_a if a else ps_b
            cap = 1536 if a else 1024
            pl = pool.tile([128, cap], F32, tag="pl" + ("A" if a else "B"),
                           name=f"pl_{s}_{start}")
            for local in range(0, size, 512):
                gcol = start + local
                kt, qh = divmod(gcol, 1024)
                lhsT = kalls[seg][:, si * KP + kt * 128: si * KP + (kt + 1) * 128]
                nc.tensor.matmul(
                    pl[:, local:local + 512],
                    lhsT, qall[:, s * S + qh:s * S + qh + 512],
                    start=True, stop=True)
            nc.scalar.activation(
                ring[:, start:start + size], pl[:, 0:size], Exp, scale=SCALE)

        def emit_av_kt(s, ring, kt, po, n_kt):
            seg = 0 if s < n_a else 1
            si = s if s < n_a else s - n_a
            VW = VWs[seg]
            first, last = kt == 0, kt == n_kt - 1
            rhs = valls[seg][:, kt * VW + si * 129: kt * VW + (si + 1) * 129]
            for qi in range(NQT):
                off = po_off(qi)
                # start=True clears the has_written bits of the WHOLE
                # bank, so only the first matmul touching each bank may
                # carry it; the other regions' first writes rely on
                # their (now cleared) bits selecting overwrite mode.
                nc.tensor.matmul(
                    po[:, off:off + 129],
                    ring[:, kt * 1024 + qi * 128: kt * 1024 + (qi + 1) * 128],
                    rhs, start=first and qi % 3 == 0, stop=last,
                    skip_group_check=True)

        def emit_epilogue(s, po, last=False):
            # Split per PSUM bank; the copies run first since they are
            # what releases the po banks for the next slot's AV matmuls.
            oal = sb_o.tile([128, 1536], F16, tag="oal", name=f"oal_{s}")
            rst = sb_o.tile([128, 9], F32, tag="rst", name=f"rst_{s}")
            osb = sb_o.tile([128, S], F16, tag="osb", name=f"osb_{s}")
            for g in range(3):
                cnt = 3 if g < 2 else NQT - 6
                base = g * 512
                nc.vector.tensor_copy(
                    oal[:, base:base + cnt * 129], po[:, base:base + cnt * 129])
            for g in range(3):
                cnt = 3 if g < 2 else NQT - 6
                base = g * 512
                nc.vector.reciprocal(
                    rst[:, g * 3:g * 3 + cnt],
                    oal[:, base + 128:base + cnt * 129:129])
            for qi in range(NQT):
                g, j = divmod(qi, 3)
                nc.vector.tensor_scalar_mul(
                    osb[:, qi * 128:(qi + 1) * 128],
                    oal[:, g * 512 + j * 129:g * 512 + j * 129 + 128],
                    rst[:, qi:qi + 1])
            nc.gpsimd.dma_start(out_t.ap()[s], osb[:])

        # Boundary-level software pipeline. Per slot s the PE queue gets:
        #   QK(s, g0) | AV(s-1, kt 0..last-1) | QK(s, g1) | AV(s-1, last)
        #   | QK(s, g2..) | epilogue(s-1)
        # so exp(s, g0) starts the moment exp(s-1, last) finishes, and the
        # previous slot's AV (whose exps are long done) fills PE idle time
        # without head-of-line blocking the QK stream.
        n_kts = [kt_a] * n_a + [kt_b] * n_b
        rings, pos, grps = {}, {}, {}
        for s in range(H):
            n_kt = n_kts[s]
            grps[s] = _exp_groups(n_kt * 1024, force_even=s < H - 1)
            rings[s] = sb_w.tile([128, n_kt * 1024], F16, tag="ring",
                                 name=f"ring_{s}")
            if s >= 1:
                pos[s - 1] = ps_o.tile([128, 1536], F32, tag="po",
                                       name=f"po_{s - 1}")
            for gi in range(len(grps[s])):
                emit_qk_group(s, gi, rings[s], grps[s])
                if s >= 1 and gi == 0:
                    for kt in range(n_kts[s - 1] - 1):
                        emit_av_kt(s - 1, rings[s - 1], kt, pos[s - 1],
                                   n_kts[s - 1])
                if s >= 1 and gi == 1:
                    emit_av_kt(s - 1, rings[s - 1], n_kts[s - 1] - 1,
                               pos[s - 1], n_kts[s - 1])
            if s >= 1:
                emit_epilogue(s - 1, pos.pop(s - 1))
                rings.pop(s - 1)
        # Last slot: its AV matmuls chase the exps straight down the queue.
        pos[H - 1] = ps_o.tile([128, 1536], F32, tag="po", name=f"po_{H - 1}")
        for kt in range(n_kts[H - 1]):
            emit_av_kt(H - 1, rings[H - 1], kt, pos[H - 1], n_kts[H - 1])
        emit_epilogue(H - 1, pos.pop(H - 1), last=True)

    nc.compile()
    return nc


def kernel(memory, query, seq_mask, b):
    memory = np.ascontiguousarray(memory, dtype=np.float32)
    query = np.ascontiguousarray(query, dtype=np.float32)
    seq_mask = np.asarray(seq_mask)
    assert memory.shape == (B, S, 2 * D) and query.shape == (B, S, D)

    counts = [int(np.count_nonzero(seq_mask[i])) for i in range(B)]
    tiles = [max((c + 127) // 128, 1) for c in counts]
    kt_a = max(tiles)
    big = [i for i in range(B) if tiles[i] == kt_a]
    if len(big) == B:
        big = big[:-1]
    small = [i for i in range(B) if i not in big]
    kt_b = max(tiles[i] for i in small)
    n_a = len(big)            # head-slots per core from "big" batches

    key = (kt_a, kt_b, n_a, tuple(sorted(OPTS.items())))
    if key not in _NC_CACHE:
        _NC_CACHE[key] = _build(kt_a, kt_b, n_a, OPTS)
    nc = _NC_CACHE[key]

    # Per-batch compacted/transposed operands (fp16).
    q_t = np.ascontiguousarray(query.transpose(0, 2, 1)).astype(np.float16)
    ktb_all, vab_all = {}, {}
    for i in range(B):
        kp = (kt_a if i in big else kt_b) * 128
        idx = np.flatnonzero(seq_mask[i])
        nb = len(idx)
        ktb = np.zeros((D, kp), dtype=np.float16)
        vab = np.zeros((kp, H, 129), dtype=np.float16)
        if nb:
            ktb[:, :nb] = memory[i, idx, :D].T
            vab[:nb, :, :128] = memory[i, idx, D:].reshape(nb, H, DH)
            vab[:nb, :, 128] = 1.0
        ktb_all[i], vab_all[i] = ktb, vab

    # Deal the (batch, head) pairs: core c gets n_a consecutive entries
    # of the "big" head list and 8-n_a of the "small" head list.
    heads_a = [(bi, h) for bi in big for h in range(H)]
    heads_b = [(bi, h) for bi in small for h in range(H)]
    n_b = H - n_a
    placements, in_maps = [], []
    for c in range(B):
        slots = heads_a[c * n_a:(c + 1) * n_a] + heads_b[c * n_b:(c + 1) * n_b]
        placements.append(slots)
        qrows = np.concatenate(
            [q_t[bi][h * DH:(h + 1) * DH] for bi, h in slots], axis=0)
        ka = np.concatenate(
            [ktb_all[bi][h * DH:(h + 1) * DH] for bi, h in slots[:n_a]], axis=0)
        kb = np.concatenate(
            [ktb_all[bi][h * DH:(h + 1) * DH] for bi, h in slots[n_a:]], axis=0)
        va = np.concatenate(
            [vab_all[bi][:, h] for bi, h in slots[:n_a]], axis=1)
        vb = np.concatenate(
            [vab_all[bi][:, h] for bi, h in slots[n_a:]], axis=1)
        in_maps.append({
            "q_t": np.ascontiguousarray(qrows),
            "k_a": np.ascontiguousarray(ka),
            "k_b": np.ascontiguousarray(kb),
            "v_a": np.ascontiguousarray(va),
            "v_b": np.ascontiguousarray(vb),
        })

    res = run_bass_kernel_spmd(nc, in_maps, list(range(B)))
    out = np.empty((B, S, D), dtype=np.float32)
    for c, slots in enumerate(placements):
        o = res.results[c]["out_t"].astype(np.float32)   # [8, 128, S]
        for j, (bi, h) in enumerate(slots):
            # [p, (qi d)] -> [qi, p, d] -> [S, d]
            blk = o[j].reshape(128, NQT, DH).transpose(1, 0, 2)
            out[bi][:, h * DH:(h + 1) * DH] = blk.reshape(S, DH)
    for i in range(B):
        if counts[i] == 0:
            # all keys masked: reference softmax degenerates to uniform
            out[i] = memory[i, :, D:].mean(axis=0)[None, :]
    return out


# revision 37
# speedup vs baseline: 1.1955x; 1.0119x over previous
"""Multi-head attention (B=8, H=8, S=1024, d=128) on 8 TRN2 NeuronCores.

Strategy
--------
- 2D sharding over (batch, head): the 64 (batch, head) attention
  problems are dealt to the 8 cores so that every core gets the same
  mix of "large-mask" and "small-mask" batches (the number of 128-wide
  key tiles surviving seq_mask compaction varies per batch, and the
  scalar engine's exp throughput is the kernel bottleneck). Each core
  runs n_a head-slots with kt_a key tiles and n_b = 8-n_a slots with
  kt_b tiles; the (kt_a, n_a, kt_b) program shape is identical on all
  cores (SPMD), only the data differs.
- Host-side prep (layout only): per batch, compact keys/values to the
  seq_mask-selected rows (zero-padded to the segment's k-tile count),
  pre-transpose Q and compacted K so the contraction dim (d) lands on
  SBUF partitions, and cast matmul operands to fp16. V is augmented per
  head with a 129th "indicator" column (1 for real keys, 0 for padding)
  so the softmax denominator falls out of the AV matmul.
- Device math per head-slot:
    logitsT[k, q] = K_h^T.T @ Q_h^T          (PE, M=128 k-tiles, N=512)
    W^T[k, q]     = exp(logitsT * d^-0.5)    (ACT, PSUM -> SBUF fp16,
                                              batched in alternating
                                              1536/1024-col groups to
                                              amortize ACTIVATE overhead)
    out[q, 129]   = sum_kt W^T[kt,qtile].T @ [V_h[kt] | ind[kt]]
                                             (PE, M=128 q-tiles, N=129,
                                              PSUM accumulation over kt;
                                              col 128 = denominator)
    osb[q, d]     = out[:, :128] * recip(out[:, 128])  (DVE)
  The learned scalar bias b cancels in softmax (shift invariance) and
  the -1e30 masking is equivalent to dropping masked keys, which the
  compaction does exactly.
- Software pipelining: AV+epilogue of slot s-1 are interleaved into the
  QK group stream of slot s so the scalar engine never waits at slot
  boundaries. A short burst of dummy matmuls at kernel start warms the
  PE HAM clock gate while the first input DMAs are in flight.
- Output per head-slot is DMA'd as a contiguous [128, 1024] fp16 block
  ([q-within-tile, (q-tile, d)]); the host reassembles [S, D] and
  handles the degenerate all-masked batch (uniform average).
"""
from contextlib import ExitStack

import numpy as np

import concourse.bacc as bacc
import concourse.mybir as mybir
import concourse.tile as tile
from concourse.bass_utils import run_bass_kernel_spmd

F32 = mybir.dt.float32
F16 = mybir.dt.float16
Exp = mybir.ActivationFunctionType.Exp

B, S, D, H = 8, 1024, 1024, 8
DH = D // H              # 128, head dim = one partition tile
SCALE = float(DH) ** -0.5
NQT = S // 128           # 8 q-tiles per head

_NC_CACHE: dict[tuple, object] = {}

# build options (overridable for profiling experiments)
OPTS: dict = {}


def _exp_groups(total, force_even=False, start_a=True):
    """Alternating 1536/1024-col exp groups covering `total` columns.

    With force_even, an odd group count ending in a full 1536 A-group is
    reshaped to end on a B-group (512-A + 1024-B) so that consecutive
    slots keep strict A/B alternation (no same-slot-pool stall at the
    slot boundary)."""
    groups, pos, a = [], 0, start_a
    while pos < total:
        size = min(1536 if a else 1024, total - pos)
        groups.append((a, pos, size))
        pos += size
        a = not a
    if force_even and len(groups) % 2 and groups[-1][2] == 1536:
        a_, p, _ = groups.pop()
        groups += [(a_, p, 512), (not a_, p + 512, 1024)]
    return groups


def _build(kt_a: int, kt_b: int, n_a: int, opts: dict | None = None):
    """Per-core kernel: n_a head-slots with kt_a k-tiles, then 8-n_a
    slots with kt_b k-tiles."""
    opts = opts or {}
    w_bufs = opts.get("w_bufs", 2)
    o_bufs = opts.get("o_bufs", 2)
    n_warm = opts.get("n_warm", 8)
    n_b = H - n_a
    KPA, KPB = kt_a * 128, kt_b * 128
    VWA, VWB = n_a * 129, n_b * 129
    nc = bacc.Bacc("TRN2", target_bir_lowering=False, debug=False)

    q_t = nc.dram_tensor("q_t", [D, S], F16, kind="ExternalInput")
    k_a = nc.dram_tensor("k_a", [n_a * DH, KPA], F16, kind="ExternalInput")
    k_b = nc.dram_tensor("k_b", [n_b * DH, KPB], F16, kind="ExternalInput")
    v_a = nc.dram_tensor("v_a", [KPA, VWA], F16, kind="ExternalInput")
    v_b = nc.dram_tensor("v_b", [KPB, VWB], F16, kind="ExternalInput")
    out_t = nc.dram_tensor("out_t", [H, 128, S], F16, kind="ExternalOutput")

    def po_off(qi):
        g, j = divmod(qi, 3)
        return g * 512 + j * 129

    with tile.TileContext(nc) as tc, ExitStack() as ctx:
        sb_k = ctx.enter_context(tc.tile_pool(name="sb_k", bufs=1))
        sb_q = ctx.enter_context(tc.tile_pool(name="sb_q", bufs=1))
        sb_v = ctx.enter_context(tc.tile_pool(name="sb_v", bufs=1))
        sb_wm = ctx.enter_context(tc.tile_pool(name="sb_wm", bufs=1))
        sb_w = ctx.enter_context(tc.tile_pool(name="sb_w", bufs=w_bufs))
        sb_o = ctx.enter_context(tc.tile_pool(name="sb_o", bufs=o_bufs))
        ps_a = ctx.enter_context(tc.tile_pool(name="ps_a", bufs=1, space="PSUM"))
        ps_b = ctx.enter_context(tc.tile_pool(name="ps_b", bufs=1, space="PSUM"))
        ps_o = ctx.enter_context(tc.tile_pool(name="ps_o", bufs=1, space="PSUM"))

        kalls = [sb_k.tile([128, n_a * KPA], F16, tag="ka", name="ka"),
                 sb_k.tile([128, n_b * KPB], F16, tag="kb", name="kb")]
        qall = sb_q.tile([128, H * S], F16)
        valls = [sb_v.tile([128, kt_a * VWA], F16, tag="va", name="va"),
                 sb_v.tile([128, kt_b * VWB], F16, tag="vb", name="vb")]
        kts = [k_a, k_b]
        vts = [v_a, v_b]
        KPs, VWs = (KPA, KPB), (VWA, VWB)

        # --- PE warmup: dense dummy matmuls while the first DMAs fly, so
        # the HAM clock gate reaches 8/8 before real work arrives.
        if n_warm:
            wl = sb_wm.tile([128, 128], F16)
            wr = sb_wm.tile([128, 512], F16)
            nc.gpsimd.memset(wl[:], 0.0)
            nc.gpsimd.memset(wr[:], 0.0)
            warm_po = ps_o.tile([128, 1536], F32, tag="po", name="po_warm")
            for _ in range(n_warm):
                nc.tensor.matmul(warm_po[:, 0:512], wl[:], wr[:],
                                 start=True, stop=True, skip_group_check=True)

        # --- Input DMAs, split per head-slot / per k-tile; the first few
        # are issued from different engines so their transfers overlap.
        def dma_k(seg, si, eng):
            KP = KPs[seg]
            eng.dma_start(kalls[seg][:, si * KP:(si + 1) * KP],
                          kts[seg].ap()[si * DH:(si + 1) * DH, :])

        def dma_q(s, eng):
            eng.dma_start(
                qall[:, s * S:(s + 1) * S], q_t.ap()[s * DH:(s + 1) * DH, :])

        def dma_v(seg, kt, eng):
            VW = VWs[seg]
            eng.dma_start(
                valls[seg][:, kt * VW:(kt + 1) * VW],
                vts[seg].ap()[kt * 128:(kt + 1) * 128, :])

        dma_q(0, nc.sync)
        dma_k(0, 0, nc.gpsimd)
        dma_k(0, 1, nc.scalar)
        dma_q(1, nc.sync)
        dma_v(0, 0, nc.scalar)
        dma_v(0, 1, nc.scalar)
        dma_q(2, nc.sync)
        for kt in range(2, kt_a):
            dma_v(0, kt, nc.gpsimd)
        for si in range(2, n_a):
            dma_k(0, si, nc.gpsimd)
            if si >= 3:
                dma_q(si, nc.sync)
        for si in range(n_b):
            dma_k(1, si, nc.gpsimd)
        for s in range(n_a, H):
            dma_q(s, nc.sync)
        for kt in range(kt_b):
            dma_v(1, kt, nc.sync)

        def emit_qk_group(s, gi, ring, groups):
            seg = 0 if s < n_a else 1
            si = s if s < n_a else s - n_a
            KP = KPs[seg]
            a, start, size = groups[gi]
            pool = ps_a if a else ps_b
            cap = 1536 if a else 1024
            pl = pool.tile([128, cap], F32, tag="pl" + ("A" if a else "B"),
                           name=f"pl_{s}_{start}")
            for local in range(0, size, 512):
                gcol = start + local
                kt, qh = divmod(gcol, 1024)
                lhsT = kalls[seg][:, si * KP + kt * 128: si * KP + (kt + 1) * 128]
                nc.tensor.matmul(
                    pl[:, local:local + 512],
                    lhsT, qall[:, s * S + qh:s * S + qh + 512],
                    start=True, stop=True)
            nc.scalar.activation(
                ring[:, start:start + size], pl[:, 0:size], Exp, scale=SCALE)

        def emit_av_kt(s, ring, kt, po, n_kt):
            seg = 0 if s < n_a else 1
            si = s if s < n_a else s - n_a
            VW = VWs[seg]
            first, last = kt == 0, kt == n_kt - 1
            rhs = valls[seg][:, kt * VW + si * 129: kt * VW + (si + 1) * 129]
            for qi in range(NQT):
                off = po_off(qi)
                # start=True clears the has_written bits of the WHOLE
                # bank, so only the first matmul touching each bank may
                # carry it; the other regions' first writes rely on
                # their (now cleared) bits selecting overwrite mode.
                nc.tensor.matmul(
                    po[:, off:off + 129],
                    ring[:, kt * 1024 + qi * 128: kt * 1024 + (qi + 1) * 128],
                    rhs, start=first and qi % 3 == 0, stop=last,
                    skip_group_check=True)

        def emit_epilogue(s, po, last=False):
            # Split per PSUM bank; the copies run first since they are
            # what releases the po banks for the next slot's AV matmuls.
            oal = sb_o.tile([128, 1536], F16, tag="oal", name=f"oal_{s}")
            rst = sb_o.tile([128, 9], F32, tag="rst", name=f"rst_{s}")
            osb = sb_o.tile([128, S], F16, tag="osb", name=f"osb_{s}")
            for g in range(3):
                cnt = 3 if g < 2 else NQT - 6
                base = g * 512
                nc.vector.tensor_copy(
                    oal[:, base:base + cnt * 129], po[:, base:base + cnt * 129])
            for g in range(3):
                cnt = 3 if g < 2 else NQT - 6
                base = g * 512
                nc.vector.reciprocal(
                    rst[:, g * 3:g * 3 + cnt],
                    oal[:, base + 128:base + cnt * 129:129])
            for qi in range(NQT):
                g, j = divmod(qi, 3)
                nc.vector.tensor_scalar_mul(
                    osb[:, qi * 128:(qi + 1) * 128],
                    oal[:, g * 512 + j * 129:g * 512 + j * 129 + 128],
                    rst[:, qi:qi + 1])
            nc.gpsimd.dma_start(out_t.ap()[s], osb[:])

        # Boundary-level software pipeline. Per slot s the PE queue gets:
        #   QK(s, g0) | AV(s-1, kt 0..last-1) | QK(s, g1) | AV(s-1, last)
        #   | QK(s, g2..) | epilogue(s-1)
        # so exp(s, g0) starts the moment exp(s-1, last) finishes, and the
        # previous slot's AV (whose exps are long done) fills PE idle time
        # without head-of-line blocking the QK stream.
        n_kts = [kt_a] * n_a + [kt_b] * n_b
        rings, pos, grps = {}, {}, {}
        for s in range(H):
            n_kt = n_kts[s]
            # slot 0 starts with the 1024 (B) group: its first exp then
            # depends only on k-tile 0 + q0, shortening the kernel head
            grps[s] = _exp_groups(n_kt * 1024, force_even=s < H - 1,
                                  start_a=s > 0)
            rings[s] = sb_w.tile([128, n_kt * 1024], F16, tag="ring",
                                 name=f"ring_{s}")
            if s >= 1:
                pos[s - 1] = ps_o.tile([128, 1536], F32, tag="po",
                                       name=f"po_{s - 1}")
            for gi in range(len(grps[s])):
                emit_qk_group(s, gi, rings[s], grps[s])
                if s >= 1 and gi == 0:
                    for kt in range(n_kts[s - 1] - 1):
                        emit_av_kt(s - 1, rings[s - 1], kt, pos[s - 1],
                                   n_kts[s - 1])
                if s >= 1 and gi == 1:
                    emit_av_kt(s - 1, rings[s - 1], n_kts[s - 1] - 1,
                               pos[s - 1], n_kts[s - 1])
            if s >= 1:
                emit_epilogue(s - 1, pos.pop(s - 1))
                rings.pop(s - 1)
        # Last slot: its AV matmuls chase the exps straight down the queue.
        pos[H - 1] = ps_o.tile([128, 1536], F32, tag="po", name=f"po_{H - 1}")
        for kt in range(n_kts[H - 1]):
            emit_av_kt(H - 1, rings[H - 1], kt, pos[H - 1], n_kts[H - 1])
        emit_epilogue(H - 1, pos.pop(H - 1), last=True)

    nc.compile()
    return nc


def kernel(memory, query, seq_mask, b):
    memory = np.ascontiguousarray(memory, dtype=np.float32)
    query = np.ascontiguousarray(query, dtype=np.float32)
    seq_mask = np.asarray(seq_mask)
    assert memory.shape == (B, S, 2 * D) and query.shape == (B, S, D)

    counts = [int(np.count_nonzero(seq_mask[i])) for i in range(B)]
    tiles = [max((c + 127) // 128, 1) for c in counts]
    kt_a = max(tiles)
    big = [i for i in range(B) if tiles[i] == kt_a]
    if len(big) == B:
        big = big[:-1]
    small = [i for i in range(B) if i not in big]
    kt_b = max(tiles[i] for i in small)
    n_a = len(big)            # head-slots per core from "big" batches

    key = (kt_a, kt_b, n_a, tuple(sorted(OPTS.items())))
    if key not in _NC_CACHE:
        _NC_CACHE[key] = _build(kt_a, kt_b, n_a, OPTS)
    nc = _NC_CACHE[key]

    # Per-batch compacted/transposed operands (fp16).
    q_t = np.ascontiguousarray(query.transpose(0, 2, 1)).astype(np.float16)
    ktb_all, vab_all = {}, {}
    for i in range(B):
        kp = (kt_a if i in big else kt_b) * 128
        idx = np.flatnonzero(seq_mask[i])
        nb = len(idx)
        ktb = np.zeros((D, kp), dtype=np.float16)
        vab = np.zeros((kp, H, 129), dtype=np.float16)
        if nb:
            ktb[:, :nb] = memory[i, idx, :D].T
            vab[:nb, :, :128] = memory[i, idx, D:].reshape(nb, H, DH)
            vab[:nb, :, 128] = 1.0
        ktb_all[i], vab_all[i] = ktb, vab

    # Deal the (batch, head) pairs: core c gets n_a consecutive entries
    # of the "big" head list and 8-n_a of the "small" head list.
    heads_a = [(bi, h) for bi in big for h in range(H)]
    heads_b = [(bi, h) for bi in small for h in range(H)]
    n_b = H - n_a
    placements, in_maps = [], []
    for c in range(B):
        slots = heads_a[c * n_a:(c + 1) * n_a] + heads_b[c * n_b:(c + 1) * n_b]
        placements.append(slots)
        qrows = np.concatenate(
            [q_t[bi][h * DH:(h + 1) * DH] for bi, h in slots], axis=0)
        ka = np.concatenate(
            [ktb_all[bi][h * DH:(h + 1) * DH] for bi, h in slots[:n_a]], axis=0)
        kb = np.concatenate(
            [ktb_all[bi][h * DH:(h + 1) * DH] for bi, h in slots[n_a:]], axis=0)
        va = np.concatenate(
            [vab_all[bi][:, h] for bi, h in slots[:n_a]], axis=1)
        vb = np.concatenate(
            [vab_all[bi][:, h] for bi, h in slots[n_a:]], axis=1)
        in_maps.append({
            "q_t": np.ascontiguousarray(qrows),
            "k_a": np.ascontiguousarray(ka),
            "k_b": np.ascontiguousarray(kb),
            "v_a": np.ascontiguousarray(va),
            "v_b": np.ascontiguousarray(vb),
        })

    res = run_bass_kernel_spmd(nc, in_maps, list(range(B)))
    out = np.empty((B, S, D), dtype=np.float32)
    for c, slots in enumerate(placements):
        o = res.results[c]["out_t"].astype(np.float32)   # [8, 128, S]
        for j, (bi, h) in enumerate(slots):
            # [p, (qi d)] -> [qi, p, d] -> [S, d]
            blk = o[j].reshape(128, NQT, DH).transpose(1, 0, 2)
            out[bi][:, h * DH:(h + 1) * DH] = blk.reshape(S, DH)
    for i in range(B):
        if counts[i] == 0:
            # all keys masked: reference softmax degenerates to uniform
            out[i] = memory[i, :, D:].mean(axis=0)[None, :]
    return out
